# Initial kernel scaffold
#
"""CQAttention (BiDAF context-query attention) Trainium2 kernel.

Shapes: C (32,128,1024), Q (32,128,512), W (32768,1,384) -> out (32,512,1024).
Data-parallel across 8 NeuronCores: 4 batches per core, no collectives.

Per-batch algorithm (all tiles d-major, 128 partitions):
  Ct   = C^T chunks (PE transpose)
  U    = wq + wqc*Ct ; r = sum_d(wc*Ct)          (DVE, c-major chunks)
  U^T  via PE transpose
  S    (c,q) = U^T.T @ Q        -> E = exp(S + r)       (ACT bias=r, accum=rowsum)
  S^T  (q,c) = Q.T @ U^T  with PSUM prefilled with (r - log rowsum) broadcast
            -> F = exp(.) = S1^T exactly (softmax over q folded into exponent)
  G    (q,d+1) = E.T @ [Ct | 1]  -> colsum in last col; Gn = G[:, :d]/colsum = S2tC
  A^T  = Qt @ F ; B^T = Gn @ F
  out  = [C ; A^T ; C*A^T ; C*B^T]
"""

import numpy as np

import concourse.bass as bass
import concourse.bacc as bacc
import concourse.mybir as mybir
from concourse import tile
from concourse.bass_utils import run_bass_kernel_spmd

B, D, CL, QL = 32, 128, 1024, 512
NCORES = 8
BPC = B // NCORES          # batches per core
NC_CHUNK = CL // D         # 8 c-chunks of 128
NQ_CHUNK = QL // D         # 4 q-chunks of 128

F32 = mybir.dt.float32
F32R = mybir.dt.float32r
BF16 = mybir.dt.bfloat16
EXP = mybir.ActivationFunctionType.Exp
LOG = mybir.ActivationFunctionType.Ln
MULT = mybir.AluOpType.mult
ADD = mybir.AluOpType.add

_NC = None


def r32(ap):
    return ap.bitcast(F32R)


def _build():
    nc = bacc.Bacc("TRN2", debug=False, num_devices=NCORES)

    C_d = nc.dram_tensor("C", [BPC, D, CL], F32, kind="ExternalInput").ap()
    Q_d = nc.dram_tensor("Q", [BPC, D, QL], F32, kind="ExternalInput").ap()
    W_d = nc.dram_tensor("W", [BPC, CL, 3 * D], F32, kind="ExternalInput").ap()
    EYE_d = nc.dram_tensor("EYE", [D, D], F32, kind="ExternalInput").ap()
    ONE_d = nc.dram_tensor("ONE", [1, D], F32, kind="ExternalInput").ap()
    OUT_d = nc.dram_tensor("OUT", [BPC, 4 * D, CL], F32, kind="ExternalOutput").ap()

    with tile.TileContext(nc) as tc:
        with (
            tc.tile_pool(name="const", bufs=1) as cpool,
            tc.tile_pool(name="work", bufs=2) as pool,
            tc.tile_pool(name="psT", bufs=2, space="PSUM") as psT,
            tc.tile_pool(name="psS", bufs=3, space="PSUM") as psS,
            tc.tile_pool(name="psAB", bufs=2, space="PSUM") as psAB,
            tc.tile_pool(name="psG", bufs=1, space="PSUM") as psG,
            tc.tile_pool(name="dram", bufs=2, space="DRAM") as dram,
        ):
            eye = cpool.tile([D, D], F32)
            ones_row = cpool.tile([1, D], F32)
            nc.sync.dma_start(r32(eye[:]), r32(EYE_d[:]))
            nc.sync.dma_start(r32(ones_row[:]), r32(ONE_d[:]))

            pools = (pool, psT, psS, psAB, psG, dram)
            for b in range(BPC):
                _batch(nc, tc, pools, eye, ones_row,
                       C_d[b], Q_d[b], W_d[b], OUT_d[b])
    nc.compile()
    return nc


def _batch(nc, tc, pools, eye, ones_row, C_d, Q_d, W_d, OUT_d):
    pool, psT, psS, psAB, psG, dram = pools
    # ---- loads ----
    Ctile = pool.tile([D, CL], F32, tag="Ctile", bufs=3)
    Qtile = pool.tile([D, QL], F32, tag="Qtile", bufs=3)
    Wtile = pool.tile([D, NC_CHUNK * 3 * D], F32, tag="Wtile")
    nc.sync.dma_start(r32(Ctile[:]), r32(C_d[:]))
    nc.sync.dma_start(r32(Qtile[:]), r32(Q_d[:]))
    # W (CL, 3D) -> (128, k, 3D): chunk-major per-partition layout
    nc.sync.dma_start(
        Wtile.rearrange("p (k e) -> p k e", k=NC_CHUNK),
        W_d.rearrange("(k p) e -> p k e", p=D),
    )

    # ---- Ct: transpose C chunks; Ct (128, 1024) f32 + CtOnes (128, 8*129) bf16
    Ct = pool.tile([D, CL], F32, tag="Ct", bufs=3)
    CtOnes = pool.tile([D, NC_CHUNK * (D + 1)], BF16, tag="CtOnes", bufs=3)
    co_view = CtOnes.rearrange("p (k d) -> p k d", k=NC_CHUNK)
    nc.vector.memset(co_view[:, :, D:D + 1], 1.0)
    for g in range(2):  # two groups of 4 chunks per PSUM bank
        ps = psT.tile([D, 4 * D], F32, tag="ps")
        for i in range(4):
            k = 4 * g + i
            nc.tensor.transpose(r32(ps[:, i * D:(i + 1) * D]),
                                r32(Ctile[:, k * D:(k + 1) * D]), r32(eye[:]))
        nc.scalar.copy(Ct[:, g * 4 * D:(g + 1) * 4 * D], ps[:])
        nc.vector.tensor_copy(co_view[:, 4 * g:4 * g + 4, 0:D], ps.rearrange("p (i d) -> p i d", i=4))

    # ---- U = wq + wqc*Ct ; r = sum_d(wc * Ct) ----
    U = pool.tile([D, CL], F32, tag="U", bufs=3)
    rbias = pool.tile([D, NC_CHUNK], F32, tag="rbias", bufs=3)
    rscr = pool.tile([D, CL], F32, tag="rscr", bufs=1)
    w_view = Wtile.rearrange("p (k e) -> p k e", k=NC_CHUNK)
    u_view = U.rearrange("p (k d) -> p k d", k=NC_CHUNK)
    ct_view = Ct.rearrange("p (k d) -> p k d", k=NC_CHUNK)
    nc.vector.tensor_mul(r32(u_view[:]), w_view[:, :, 2 * D:3 * D], ct_view[:])
    nc.vector.tensor_add(r32(u_view[:]), u_view[:], w_view[:, :, 0:D])
    rscr_view = rscr.rearrange("p (k d) -> p k d", k=NC_CHUNK)
    nc.vector.tensor_mul(rscr_view[:], w_view[:, :, D:2 * D], ct_view[:])
    nc.vector.tensor_reduce(rbias[:], rscr_view[:], axis=mybir.AxisListType.X, op=ADD)

    # ---- U^T via PE transpose ----
    UT = pool.tile([D, CL], F32, tag="UT", bufs=3)
    for g in range(2):
        ps = psT.tile([D, 4 * D], F32, tag="ps")
        for i in range(4):
            k = 4 * g + i
            nc.tensor.transpose(r32(ps[:, i * D:(i + 1) * D]),
                                r32(U[:, k * D:(k + 1) * D]), r32(eye[:]))
        nc.scalar.copy(r32(UT[:, g * 4 * D:(g + 1) * 4 * D]), ps[:])

    # ---- Qt via PE transpose ----
    Qt = pool.tile([D, QL], F32, tag="Qt", bufs=3)
    ps = psT.tile([D, 4 * D], F32, tag="ps")
    for j in range(NQ_CHUNK):
        nc.tensor.transpose(r32(ps[:, j * D:(j + 1) * D]),
                            r32(Qtile[:, j * D:(j + 1) * D]), r32(eye[:]))
    nc.scalar.copy(r32(Qt[:]), ps[:])

    # ---- S (c,q) -> E = exp(S + r), rowsum ----
    E = pool.tile([D, NC_CHUNK * QL], BF16, tag="E", bufs=3)
    rowsum = pool.tile([D, NC_CHUNK], F32, tag="rowsum", bufs=3)
    for k in range(NC_CHUNK):
        ps = psS.tile([D, QL], F32, tag="ps")
        nc.tensor.matmul(ps[:], r32(UT[:, k * D:(k + 1) * D]), r32(Qtile[:]),
                         start=True, stop=True)
        nc.scalar.activation(E[:, k * QL:(k + 1) * QL], ps[:], EXP,
                             bias=rbias[:, k:k + 1], accum_out=rowsum[:, k:k + 1])

    # ---- S^T (q,c) -> F = exp(S^T) (unnormalized; softmax-q scale applied at outputs)
    Ftile = pool.tile([D, NQ_CHUNK * CL], F32, tag="Ftile")
    for j in range(NQ_CHUNK):
        for h in range(2):
            ps = psS.tile([D, QL], F32, tag="ps")
            nc.tensor.matmul(ps[:], r32(Qtile[:, j * D:(j + 1) * D]),
                             r32(UT[:, h * QL:(h + 1) * QL]), start=True, stop=True)
            nc.scalar.activation(r32(Ftile[:, j * CL + h * QL: j * CL + (h + 1) * QL]),
                                 ps[:], EXP)

    # ---- G (q, d+1) = E.T @ [Ct | 1] ; Gn = G/colsum ----
    Gn = pool.tile([D, QL], F32, tag="Gn")
    crecip = pool.tile([D, NQ_CHUNK], F32, tag="crecip")
    for j in range(NQ_CHUNK):
        psg = psG.tile([D, D + 1], F32, tag="psg")
        for k in range(NC_CHUNK):
            nc.tensor.matmul(psg[:], E[:, k * QL + j * D: k * QL + (j + 1) * D],
                             CtOnes[:, k * (D + 1):(k + 1) * (D + 1)],
                             start=(k == 0), stop=(k == NC_CHUNK - 1))
        nc.vector.reciprocal(crecip[:, j:j + 1], psg[:, D:D + 1])
        nc.vector.tensor_scalar_mul(r32(Gn[:, j * D:(j + 1) * D]), psg[:, 0:D],
                                    crecip[:, j:j + 1])

    # ---- rr0[c] = 1/sum_q exp(S[c,q]) = exp(r)/rowsum; broadcast to rrB (d, c).
    # Partition->row scatter bounced through DRAM scratch, then a partition-
    # replicating DMA broadcast. Both DMAs ride the ACT HWDGE ring so they are
    # never FIFO-queued behind bulk input/output traffic, and no PE instruction
    # depends on them (rrB is consumed by DVE only).
    er = pool.tile([D, NC_CHUNK], F32, tag="er", bufs=3)
    rs_inv = pool.tile([D, NC_CHUNK], F32, tag="rs_inv", bufs=3)
    rr0 = pool.tile([D, NC_CHUNK], F32, tag="rr0", bufs=3)
    rrB = pool.tile([D, CL], F32, tag="rrB")
    nc.scalar.activation(er[:], rbias[:], EXP)
    nc.vector.reciprocal(rs_inv[:], rowsum[:])
    nc.vector.tensor_mul(rr0[:], er[:], rs_inv[:])
    scratch = dram.tile([NC_CHUNK, D], F32, tag="scratch")
    nc.scalar.dma_start(scratch.rearrange("k p -> p k"), rr0[:])
    nc.scalar.dma_start(rrB[:], scratch.rearrange("k p -> (k p)")[None, :].partition_broadcast(D))

    # ---- A^T = (Qt @ F) * rrB ; B^T = (Gn @ F) * rrB ; outputs ----
    Asb = pool.tile([D, CL], F32, tag="Asb")
    CA = pool.tile([D, CL], F32, tag="CA")
    CB = pool.tile([D, CL], F32, tag="CB")
    for h in range(2):
        psa = psAB.tile([D, QL], F32, tag="ps")
        for j in range(NQ_CHUNK):
            nc.tensor.matmul(psa[:], r32(Qt[:, j * D:(j + 1) * D]),
                             r32(Ftile[:, j * CL + h * QL: j * CL + (h + 1) * QL]),
                             start=(j == 0), stop=(j == NQ_CHUNK - 1))
        nc.vector.tensor_mul(Asb[:, h * QL:(h + 1) * QL], psa[:],
                             rrB[:, h * QL:(h + 1) * QL])
        nc.vector.tensor_mul(CA[:, h * QL:(h + 1) * QL], Ctile[:, h * QL:(h + 1) * QL],
                             Asb[:, h * QL:(h + 1) * QL])
        psb = psAB.tile([D, QL], F32, tag="ps")
        for j in range(NQ_CHUNK):
            nc.tensor.matmul(psb[:], r32(Gn[:, j * D:(j + 1) * D]),
                             r32(Ftile[:, j * CL + h * QL: j * CL + (h + 1) * QL]),
                             start=(j == 0), stop=(j == NQ_CHUNK - 1))
        nc.vector.tensor_mul(CB[:, h * QL:(h + 1) * QL], psb[:],
                             rrB[:, h * QL:(h + 1) * QL])
        nc.vector.tensor_mul(CB[:, h * QL:(h + 1) * QL], CB[:, h * QL:(h + 1) * QL],
                             Ctile[:, h * QL:(h + 1) * QL])

    nc.sync.dma_start(OUT_d[0 * D:1 * D, :], Ctile[:])
    nc.sync.dma_start(OUT_d[1 * D:2 * D, :], Asb[:])
    nc.sync.dma_start(OUT_d[2 * D:3 * D, :], CA[:])
    nc.sync.dma_start(OUT_d[3 * D:4 * D, :], CB[:])


def _get_nc():
    global _NC
    if _NC is None:
        _NC = _build()
    return _NC


def kernel(C, Q, W):
    C = np.ascontiguousarray(np.asarray(C, dtype=np.float32))
    Q = np.ascontiguousarray(np.asarray(Q, dtype=np.float32))
    W = np.ascontiguousarray(np.asarray(W, dtype=np.float32)).reshape(B, CL, 3 * D)
    eye = np.eye(D, dtype=np.float32)
    ones = np.ones((1, D), dtype=np.float32)
    in_maps = [
        {
            "C": C[i * BPC:(i + 1) * BPC],
            "Q": Q[i * BPC:(i + 1) * BPC],
            "W": W[i * BPC:(i + 1) * BPC],
            "EYE": eye,
            "ONE": ones,
        }
        for i in range(NCORES)
    ]
    nc = _get_nc()
    res = run_bass_kernel_spmd(nc, in_maps, core_ids=list(range(NCORES)))
    out = np.concatenate([res.results[i]["OUT"] for i in range(NCORES)], axis=0)
    return out



# revision 1
# speedup vs baseline: 1.1089x; 1.1089x over previous
"""CQAttention (BiDAF context-query attention) Trainium2 kernel.

Shapes: C (32,128,1024), Q (32,128,512), W (32768,1,384) -> out (32,512,1024).
Data-parallel across 8 NeuronCores: 4 batches per core, no collectives.

Per-batch algorithm (all tiles d-major, 128 partitions):
  Ct   = C^T chunks (PE transpose)
  U    = wq + wqc*Ct ; r = sum_d(wc*Ct)          (DVE, c-major chunks)
  U^T  via PE transpose
  S    (c,q) = U^T.T @ Q        -> E = exp(S + r)       (ACT bias=r, accum=rowsum)
  S^T  (q,c) = Q.T @ U^T  with PSUM prefilled with (r - log rowsum) broadcast
            -> F = exp(.) = S1^T exactly (softmax over q folded into exponent)
  G    (q,d+1) = E.T @ [Ct | 1]  -> colsum in last col; Gn = G[:, :d]/colsum = S2tC
  A^T  = Qt @ F ; B^T = Gn @ F
  out  = [C ; A^T ; C*A^T ; C*B^T]
"""

import numpy as np

import concourse.bass as bass
import concourse.bacc as bacc
import concourse.mybir as mybir
from concourse import tile
from concourse.bass_utils import run_bass_kernel_spmd

B, D, CL, QL = 32, 128, 1024, 512
NCORES = 8
BPC = B // NCORES          # batches per core
NC_CHUNK = CL // D         # 8 c-chunks of 128
NQ_CHUNK = QL // D         # 4 q-chunks of 128

F32 = mybir.dt.float32
F32R = mybir.dt.float32r
BF16 = mybir.dt.bfloat16
EXP = mybir.ActivationFunctionType.Exp
LOG = mybir.ActivationFunctionType.Ln
MULT = mybir.AluOpType.mult
ADD = mybir.AluOpType.add

_NC = None


def r32(ap):
    return ap.bitcast(F32R)


def _build():
    nc = bacc.Bacc("TRN2", debug=False, num_devices=NCORES)

    C_d = nc.dram_tensor("C", [BPC, D, CL], F32, kind="ExternalInput").ap()
    Q_d = nc.dram_tensor("Q", [BPC, D, QL], F32, kind="ExternalInput").ap()
    W_d = nc.dram_tensor("W", [BPC, CL, 3 * D], F32, kind="ExternalInput").ap()
    EYE_d = nc.dram_tensor("EYE", [D, D], F32, kind="ExternalInput").ap()
    ONE_d = nc.dram_tensor("ONE", [1, D], F32, kind="ExternalInput").ap()
    OUT_d = nc.dram_tensor("OUT", [BPC, 4 * D, CL], F32, kind="ExternalOutput").ap()

    with tile.TileContext(nc) as tc:
        with (
            tc.tile_pool(name="const", bufs=1) as cpool,
            tc.tile_pool(name="work", bufs=2) as pool,
            tc.tile_pool(name="psT", bufs=2, space="PSUM") as psT,
            tc.tile_pool(name="psS", bufs=3, space="PSUM") as psS,
            tc.tile_pool(name="psAB", bufs=2, space="PSUM") as psAB,
            tc.tile_pool(name="psG", bufs=1, space="PSUM") as psG,
            tc.tile_pool(name="dram", bufs=2, space="DRAM") as dram,
        ):
            eye = cpool.tile([D, D], F32)
            ones_row = cpool.tile([1, D], F32)
            nc.sync.dma_start(r32(eye[:]), r32(EYE_d[:]))
            nc.sync.dma_start(r32(ones_row[:]), r32(ONE_d[:]))

            pools = (pool, psT, psS, psAB, psG, dram)
            for b in range(BPC):
                _batch(nc, tc, pools, eye, ones_row,
                       C_d[b], Q_d[b], W_d[b], OUT_d[b])
    nc.compile()
    return nc


def _batch(nc, tc, pools, eye, ones_row, C_d, Q_d, W_d, OUT_d):
    pool, psT, psS, psAB, psG, dram = pools
    # ---- loads ----
    Ctile = pool.tile([D, CL], F32, tag="Ctile", bufs=3)
    Qtile = pool.tile([D, QL], F32, tag="Qtile", bufs=3)
    Wtile = pool.tile([D, NC_CHUNK * 3 * D], F32, tag="Wtile")
    nc.sync.dma_start(r32(Ctile[:]), r32(C_d[:]))
    nc.sync.dma_start(r32(Qtile[:]), r32(Q_d[:]))
    # W (CL, 3D) -> (128, k, 3D): chunk-major per-partition layout
    nc.sync.dma_start(
        Wtile.rearrange("p (k e) -> p k e", k=NC_CHUNK),
        W_d.rearrange("(k p) e -> p k e", p=D),
    )

    # ---- Ct: transpose C chunks; Ct (128, 1024) f32 + CtOnes (128, 8*129) bf16
    Ct = pool.tile([D, CL], F32, tag="Ct", bufs=3)
    CtOnes = pool.tile([D, NC_CHUNK * (D + 1)], BF16, tag="CtOnes", bufs=3)
    co_view = CtOnes.rearrange("p (k d) -> p k d", k=NC_CHUNK)
    nc.vector.memset(co_view[:, :, D:D + 1], 1.0)
    for g in range(2):  # two groups of 4 chunks per PSUM bank
        ps = psT.tile([D, 4 * D], F32, tag="ps")
        for i in range(4):
            k = 4 * g + i
            nc.tensor.transpose(r32(ps[:, i * D:(i + 1) * D]),
                                r32(Ctile[:, k * D:(k + 1) * D]), r32(eye[:]))
        nc.scalar.copy(Ct[:, g * 4 * D:(g + 1) * 4 * D], ps[:])
        nc.vector.tensor_copy(co_view[:, 4 * g:4 * g + 4, 0:D], ps.rearrange("p (i d) -> p i d", i=4))

    # ---- U = wq + wqc*Ct ; r = sum_d(wc * Ct) ----
    U = pool.tile([D, CL], F32, tag="U", bufs=3)
    rbias = pool.tile([D, NC_CHUNK], F32, tag="rbias", bufs=3)
    rscr = pool.tile([D, CL], F32, tag="rscr", bufs=1)
    w_view = Wtile.rearrange("p (k e) -> p k e", k=NC_CHUNK)
    u_view = U.rearrange("p (k d) -> p k d", k=NC_CHUNK)
    ct_view = Ct.rearrange("p (k d) -> p k d", k=NC_CHUNK)
    nc.vector.tensor_mul(r32(u_view[:]), w_view[:, :, 2 * D:3 * D], ct_view[:])
    nc.vector.tensor_add(r32(u_view[:]), u_view[:], w_view[:, :, 0:D])
    rscr_view = rscr.rearrange("p (k d) -> p k d", k=NC_CHUNK)
    nc.vector.tensor_mul(rscr_view[:], w_view[:, :, D:2 * D], ct_view[:])
    nc.vector.tensor_reduce(rbias[:], rscr_view[:], axis=mybir.AxisListType.X, op=ADD)

    # ---- U^T via PE transpose ----
    UT = pool.tile([D, CL], F32, tag="UT", bufs=3)
    for g in range(2):
        ps = psT.tile([D, 4 * D], F32, tag="ps")
        for i in range(4):
            k = 4 * g + i
            nc.tensor.transpose(r32(ps[:, i * D:(i + 1) * D]),
                                r32(U[:, k * D:(k + 1) * D]), r32(eye[:]))
        nc.scalar.copy(r32(UT[:, g * 4 * D:(g + 1) * 4 * D]), ps[:])

    # ---- Qt via PE transpose ----
    Qt = pool.tile([D, QL], F32, tag="Qt", bufs=3)
    ps = psT.tile([D, 4 * D], F32, tag="ps")
    for j in range(NQ_CHUNK):
        nc.tensor.transpose(r32(ps[:, j * D:(j + 1) * D]),
                            r32(Qtile[:, j * D:(j + 1) * D]), r32(eye[:]))
    nc.scalar.copy(r32(Qt[:]), ps[:])

    # ---- S (c,q) -> E = exp(S + r), rowsum ----
    E = pool.tile([D, NC_CHUNK * QL], BF16, tag="E", bufs=3)
    rowsum = pool.tile([D, NC_CHUNK], F32, tag="rowsum", bufs=3)
    for k in range(NC_CHUNK):
        ps = psS.tile([D, QL], F32, tag="ps")
        nc.tensor.matmul(ps[:], r32(UT[:, k * D:(k + 1) * D]), r32(Qtile[:]),
                         start=True, stop=True)
        nc.scalar.activation(E[:, k * QL:(k + 1) * QL], ps[:], EXP,
                             bias=rbias[:, k:k + 1], accum_out=rowsum[:, k:k + 1])

    # ---- S^T (q,c) -> F = exp(S^T) (unnormalized; softmax-q scale applied at outputs)
    Ftile = pool.tile([D, NQ_CHUNK * CL], F32, tag="Ftile")
    for j in range(NQ_CHUNK):
        for h in range(2):
            ps = psS.tile([D, QL], F32, tag="ps")
            nc.tensor.matmul(ps[:], r32(Qtile[:, j * D:(j + 1) * D]),
                             r32(UT[:, h * QL:(h + 1) * QL]), start=True, stop=True)
            nc.scalar.activation(r32(Ftile[:, j * CL + h * QL: j * CL + (h + 1) * QL]),
                                 ps[:], EXP)

    # ---- G (q, d+1) = E.T @ [Ct | 1] ; Gn = G/colsum ----
    Gn = pool.tile([D, QL], F32, tag="Gn")
    crecip = pool.tile([D, NQ_CHUNK], F32, tag="crecip")
    for j in range(NQ_CHUNK):
        psg = psG.tile([D, D + 1], F32, tag="psg")
        for k in range(NC_CHUNK):
            nc.tensor.matmul(psg[:], E[:, k * QL + j * D: k * QL + (j + 1) * D],
                             CtOnes[:, k * (D + 1):(k + 1) * (D + 1)],
                             start=(k == 0), stop=(k == NC_CHUNK - 1))
        nc.vector.reciprocal(crecip[:, j:j + 1], psg[:, D:D + 1])
        nc.vector.tensor_scalar_mul(r32(Gn[:, j * D:(j + 1) * D]), psg[:, 0:D],
                                    crecip[:, j:j + 1])

    # ---- rr0[c] = 1/sum_q exp(S[c,q]) = exp(r)/rowsum; broadcast to rrB (d, c).
    # Partition->row scatter bounced through DRAM scratch, then a partition-
    # replicating DMA broadcast. Both DMAs ride the ACT HWDGE ring so they are
    # never FIFO-queued behind bulk input/output traffic, and no PE instruction
    # depends on them (rrB is consumed by DVE only).
    er = pool.tile([D, NC_CHUNK], F32, tag="er", bufs=3)
    rs_inv = pool.tile([D, NC_CHUNK], F32, tag="rs_inv", bufs=3)
    rr0 = pool.tile([D, NC_CHUNK], F32, tag="rr0", bufs=3)
    rrB = pool.tile([D, CL], F32, tag="rrB")
    nc.scalar.activation(er[:], rbias[:], EXP)
    nc.vector.reciprocal(rs_inv[:], rowsum[:])
    nc.vector.tensor_mul(rr0[:], er[:], rs_inv[:])
    scratch = dram.tile([NC_CHUNK, D], F32, tag="scratch")
    nc.scalar.dma_start(scratch.rearrange("k p -> p k"), rr0[:])
    nc.scalar.dma_start(rrB[:], scratch.rearrange("k p -> (k p)")[None, :].partition_broadcast(D))

    # ---- A^T = (Qt @ F) * rrB ; B^T = (Gn @ F) * rrB ; outputs ----
    Asb = pool.tile([D, CL], F32, tag="Asb")
    CA = pool.tile([D, CL], F32, tag="CA")
    CB = pool.tile([D, CL], F32, tag="CB")
    for h in range(2):
        psa = psAB.tile([D, QL], F32, tag="ps")
        for j in range(NQ_CHUNK):
            nc.tensor.matmul(psa[:], r32(Qt[:, j * D:(j + 1) * D]),
                             r32(Ftile[:, j * CL + h * QL: j * CL + (h + 1) * QL]),
                             start=(j == 0), stop=(j == NQ_CHUNK - 1))
        nc.vector.tensor_mul(Asb[:, h * QL:(h + 1) * QL], psa[:],
                             rrB[:, h * QL:(h + 1) * QL])
        nc.vector.tensor_mul(CA[:, h * QL:(h + 1) * QL], Ctile[:, h * QL:(h + 1) * QL],
                             Asb[:, h * QL:(h + 1) * QL])
        psb = psAB.tile([D, QL], F32, tag="ps")
        for j in range(NQ_CHUNK):
            nc.tensor.matmul(psb[:], r32(Gn[:, j * D:(j + 1) * D]),
                             r32(Ftile[:, j * CL + h * QL: j * CL + (h + 1) * QL]),
                             start=(j == 0), stop=(j == NQ_CHUNK - 1))
        nc.vector.tensor_mul(CB[:, h * QL:(h + 1) * QL], psb[:],
                             rrB[:, h * QL:(h + 1) * QL])
        nc.vector.tensor_mul(CB[:, h * QL:(h + 1) * QL], CB[:, h * QL:(h + 1) * QL],
                             Ctile[:, h * QL:(h + 1) * QL])

    nc.sync.dma_start(OUT_d[0 * D:1 * D, :], Ctile[:])
    nc.sync.dma_start(OUT_d[1 * D:2 * D, :], Asb[:])
    nc.sync.dma_start(OUT_d[2 * D:3 * D, :], CA[:])
    nc.sync.dma_start(OUT_d[3 * D:4 * D, :], CB[:])


def _get_nc():
    global _NC
    if _NC is None:
        _NC = _build()
    return _NC


def kernel(C, Q, W):
    C = np.ascontiguousarray(np.asarray(C, dtype=np.float32))
    Q = np.ascontiguousarray(np.asarray(Q, dtype=np.float32))
    W = np.ascontiguousarray(np.asarray(W, dtype=np.float32)).reshape(B, CL, 3 * D)
    eye = np.eye(D, dtype=np.float32)
    ones = np.ones((1, D), dtype=np.float32)
    in_maps = [
        {
            "C": C[i * BPC:(i + 1) * BPC],
            "Q": Q[i * BPC:(i + 1) * BPC],
            "W": W[i * BPC:(i + 1) * BPC],
            "EYE": eye,
            "ONE": ones,
        }
        for i in range(NCORES)
    ]
    nc = _get_nc()
    res = run_bass_kernel_spmd(nc, in_maps, core_ids=list(range(NCORES)))
    out = np.concatenate([res.results[i]["OUT"] for i in range(NCORES)], axis=0)
    return out

